# revision 3
# baseline (speedup 1.0000x reference)
"""SupCon loss kernel for Trainium2 (8 NeuronCores, SPMD row-sharded), v2.

Math (matches the reference):
  S = (E @ E^T) / T,  T = 0.1
  pos_term_i = mean_{j != i, lab_j = lab_i} S_ij
  lse_i      = logsumexp_{j != i} S_ij
  loss       = -sum_i (pos_term_i - lse_i) / N * T

v2 design (per core c, owning rows c*1024 .. c*1024+1023):
  - E pre-scaled by sqrt(1/T) and quantized to fp8e4 (TRN FP8_EXP4,
    max 240; our values are ~N(0, 3.16), rel loss err ~1e-3 vs 2e-2 gate).
  - Column-rotated E^T per core (own rows first) so the program is pure
    SPMD; stored as 8 segment tiles [128, KC=4, 1024] fp8 + G tile.
  - PE: DoubleRow fp8 matmuls (2 k-subtiles per instruction, 0.5 cyc/row)
    into [128, 2048] psum segs (4 banks, 512-col slices per matmul).
  - Diagonal self-mask via a tiny bf16 matmul (cI)^T @ (-cI) that adds
    -c^2 on the diagonal of the own-block in psum (no DVE pass needed).
  - DVE: one tensor_reduce(max, negate) per seg -> -segmax (exp bias).
  - ACT: one big ACTIVATE(Exp) per seg reading all 4 psum banks, with
    accum_out giving the seg sumexp; per-tile combine rescales segs to
    the row max (online-softmax accounting).
  - pos term: E_local @ (G/2) in fp8 (G = per-class embedding sums) into
    a psum corner before the S loop; host-prepped weights fold in 2/cnt
    and the self-dot correction.
  - Output: per-row [128, MT, 2]: (posacc - posb - rowmax, sumexp).
Host sums the 8 partial outputs -> loss = -total / N * T.
"""

import os
import sys

import numpy as np

for _p in (
    "/root/.axon_site",
    "/root/.axon_site/_ro/trn_rl_repo",
    "/root/.axon_site/_ro/pypackages",
    "/opt/trn_rl_repo",
):
    if os.path.isdir(_p) and _p not in sys.path:
        sys.path.append(_p)

import ml_dtypes

N, D, NCLS, NCORES = 8192, 512, 16, 8
ROWS = N // NCORES        # 1024 rows per core
MT = ROWS // 128          # 8 m-tiles per core
TEMP = 0.1
SCALE = 1.0 / TEMP        # 10.0
KC = D // 128             # 4 k-subtiles
SEG = 2048                # psum segment width (4 banks)
NSEG = N // SEG           # 4
DSEG = 1024               # DMA segment width
NDSEG = N // DSEG         # 8
MASKC = 8192.0            # diag mask: -MASKC^2 = -6.7e7 added on diagonal

USE_DR = True             # DoubleRow fp8 perf mode

_PROG: dict = {}


def _build_program():
    if "nc" in _PROG:
        return _PROG["nc"]

    import concourse.tile as tile
    from concourse import bacc, mybir

    dt = mybir.dt
    Alu = mybir.AluOpType
    Act = mybir.ActivationFunctionType
    f32, bf16, fp8 = dt.float32, dt.bfloat16, dt.float8e4
    DR = mybir.MatmulPerfMode.DoubleRow if USE_DR else None

    nc = bacc.Bacc("TRN2", target_bir_lowering=False, debug=False)

    et_d = nc.dram_tensor("et8", [128, KC, N], fp8, kind="ExternalInput").ap()
    g_d = nc.dram_tensor("g8", [128, KC, NCLS], fp8, kind="ExternalInput").ap()
    mask_d = nc.dram_tensor("maskI", [128, 256], bf16, kind="ExternalInput").ap()
    posw_d = nc.dram_tensor("posw", [128, MT, NCLS], f32, kind="ExternalInput").ap()
    posb_d = nc.dram_tensor("posb", [128, MT], f32, kind="ExternalInput").ap()
    out_d = nc.dram_tensor("out_vals", [128, MT, 2], f32, kind="ExternalOutput").ap()

    with tile.TileContext(nc) as tc:
        with (
            tc.tile_pool(name="ets", bufs=1) as ets,
            tc.tile_pool(name="consts", bufs=1) as consts,
            tc.tile_pool(name="small", bufs=2) as small,
            tc.tile_pool(name="dump", bufs=2) as dump,
            tc.tile_pool(name="acc", bufs=1) as accp,
            tc.tile_pool(name="psum", bufs=2, space="PSUM") as psum,
        ):
            # ---- DMA: G + seg0 first (pos matmuls + first S seg), then rest
            gt = ets.tile([128, KC, NCLS], fp8, name="g8t")
            nc.gpsimd.dma_start(gt[:], g_d[:])
            ets_seg = []
            for s in range(NDSEG):
                st = ets.tile([128, KC, DSEG], fp8, name=f"et_{s}")
                eng = nc.sync if s % 2 == 0 else nc.gpsimd
                eng.dma_start(st[:], et_d[:, :, s * DSEG : (s + 1) * DSEG])
                ets_seg.append(st)

            maskc = consts.tile([128, 256], bf16)
            nc.sync.dma_start(maskc[:], mask_d[:])
            posw = consts.tile([128, MT, NCLS], f32)
            nc.sync.dma_start(posw[:], posw_d[:])
            posb = consts.tile([128, MT], f32)
            nc.sync.dma_start(posb[:], posb_d[:])

            vals = accp.tile([128, MT, 2], f32)
            posc = accp.tile([128, 128], f32)

            # ---- pos matmuls: C[:, t*16+l] = <e_(t,:), G_l/2>, fp8 1x mode
            ps0 = psum.tile([128, SEG], f32, tag="ps")
            for t in range(MT):
                lt = t * 128
                for k in range(KC):
                    nc.tensor.matmul(
                        ps0[:, t * NCLS : (t + 1) * NCLS],
                        ets_seg[0][:, k, lt : lt + 128],
                        gt[:, k, :],
                        start=(k == 0),
                        stop=(k == KC - 1),
                    )
            nc.vector.tensor_copy(posc[:], ps0[:, 0:128])

            # ---- main loop over m-tiles
            for t in range(MT):
                nm4 = small.tile([128, NSEG], f32, tag="nm")
                nmh = small.tile([128, NSEG, 2], f32, tag="nmh")
                se4 = small.tile([128, NSEG], f32, tag="se")
                lt = t * 128
                dchunk = t // 4          # chunk of seg0 containing the diag

                for q in range(NSEG):
                    ps = psum.tile([128, SEG], f32, tag="ps")
                    if USE_DR:
                        for a in range(2):
                            lhsT = ets_seg[0][:, 2 * a : 2 * a + 2, lt : lt + 128]
                            for j in range(4):
                                c0 = q * SEG + j * 512
                                s, off = c0 // DSEG, c0 % DSEG
                                rhs = ets_seg[s][:, 2 * a : 2 * a + 2, off : off + 512]
                                nc.tensor.matmul(
                                    ps[:, j * 512 : (j + 1) * 512],
                                    lhsT,
                                    rhs,
                                    start=(a == 0),
                                    stop=(a == 1),
                                    perf_mode=DR,
                                )
                                if q == 0 and a == 1 and j == dchunk:
                                    # add -MASKC^2 on the self-diagonal
                                    nc.tensor.matmul(
                                        ps[:, lt : lt + 128],
                                        maskc[:, 0:128],
                                        maskc[:, 128:256],
                                        start=False,
                                        stop=False,
                                        skip_group_check=True,
                                    )
                    else:
                        for k in range(KC):
                            lhsT = ets_seg[0][:, k, lt : lt + 128]
                            for j in range(4):
                                c0 = q * SEG + j * 512
                                s, off = c0 // DSEG, c0 % DSEG
                                rhs = ets_seg[s][:, k, off : off + 512]
                                nc.tensor.matmul(
                                    ps[:, j * 512 : (j + 1) * 512],
                                    lhsT,
                                    rhs,
                                    start=(k == 0),
                                    stop=(k == KC - 1),
                                )
                                if q == 0 and k == KC - 1 and j == dchunk:
                                    nc.tensor.matmul(
                                        ps[:, lt : lt + 128],
                                        maskc[:, 0:128],
                                        maskc[:, 128:256],
                                        start=False,
                                        stop=False,
                                        skip_group_check=True,
                                    )

                    # half-seg maxes overlap the PE fill of the later chunks
                    for h in range(2):
                        nc.vector.tensor_reduce(
                            nmh[:, q, h : h + 1], ps[:, h * 1024 : (h + 1) * 1024],
                            axis=mybir.AxisListType.X, op=Alu.max, negate=True,
                        )
                    nc.vector.tensor_reduce(
                        nm4[:, q : q + 1], nmh[:, q, :],
                        axis=mybir.AxisListType.X, op=Alu.min,
                    )
                    expd = dump.tile([128, SEG], bf16, tag="expd")
                    nc.scalar.activation(
                        expd[:], ps[:], Act.Exp, bias=nm4[:, q : q + 1],
                        scale=1.0, accum_out=se4[:, q : q + 1],
                    )

                # combine: sumexp = sum_q se_q * exp(segmax_q - rowmax)
                negb = small.tile([128, 1], f32, tag="negb")     # -rowmax
                nc.vector.tensor_reduce(
                    negb[:], nm4[:], axis=mybir.AxisListType.X, op=Alu.min
                )
                e4 = small.tile([128, NSEG], f32, tag="e4")
                nc.scalar.activation(
                    e4[:], nm4[:], Act.Exp, bias=negb[:], scale=-1.0
                )
                t4 = small.tile([128, NSEG], f32, tag="t4")
                nc.vector.scalar_tensor_tensor(
                    out=t4[:], in0=se4[:], scalar=1.0, in1=e4[:],
                    op0=Alu.mult, op1=Alu.mult,
                    accum_out=vals[:, t, 1:2],
                )

                pos16 = small.tile([128, NCLS], f32, tag="pos16")
                posacc = small.tile([128, 1], f32, tag="posacc")
                nc.vector.scalar_tensor_tensor(
                    out=pos16[:], in0=posc[:, t * NCLS : (t + 1) * NCLS],
                    scalar=1.0, in1=posw[:, t, :],
                    op0=Alu.mult, op1=Alu.mult,
                    accum_out=posacc[:],
                )
                v1 = small.tile([128, 1], f32, tag="v1")
                nc.vector.tensor_sub(v1[:], posacc[:], posb[:, t : t + 1])
                nc.vector.tensor_add(vals[:, t, 0:1], v1[:], negb[:])

            nc.sync.dma_start(out_d[:], vals[:])

    nc.compile()
    _PROG["nc"] = nc
    return nc


def _prep_inputs(embeddings: np.ndarray, labels: np.ndarray):
    E = np.asarray(embeddings, dtype=np.float32)
    lab = np.asarray(labels).astype(np.int64)
    assert E.shape == (N, D) and lab.shape == (N,)

    # pre-scale by sqrt(1/T) so PSUM dots are already in S-units
    E8 = (E * np.float32(np.sqrt(SCALE))).astype(ml_dtypes.float8_e4m3)
    Ef = E8.astype(np.float64)

    # per-class embedding sums (halved to stay in fp8 range; posw carries 2x)
    G = np.zeros((D, NCLS), np.float64)
    for l in range(NCLS):
        G[:, l] = Ef[lab == l].sum(axis=0)
    G8 = (G / 2.0).astype(ml_dtypes.float8_e4m3)
    g8 = np.ascontiguousarray(G8.reshape(KC, 128, NCLS).transpose(1, 0, 2))

    ET = np.ascontiguousarray(E8.T)               # [D, N] fp8

    cnt = np.bincount(lab, minlength=NCLS).astype(np.float64)
    cnt_i = cnt[lab] - 1.0                        # positives per anchor
    selfdot = (Ef * Ef).sum(axis=1)               # ||e_i||^2 (S-units)
    posb_full = (selfdot / cnt_i).astype(np.float32)
    posw_full = np.zeros((N, NCLS), np.float32)
    posw_full[np.arange(N), lab] = (2.0 / cnt_i).astype(np.float32)

    maskI = np.zeros((128, 256), np.float32)
    maskI[np.arange(128), np.arange(128)] = MASKC
    maskI[np.arange(128), np.arange(128) + 128] = -MASKC
    maskI = maskI.astype(ml_dtypes.bfloat16)

    in_maps = []
    for c in range(NCORES):
        rot = np.roll(ET, -c * ROWS, axis=1)      # own columns first
        et8 = np.ascontiguousarray(
            rot.reshape(KC, 128, N).transpose(1, 0, 2)
        )
        sl = slice(c * ROWS, (c + 1) * ROWS)
        posb_c = np.ascontiguousarray(posb_full[sl].reshape(MT, 128).T)
        posw_c = np.ascontiguousarray(
            posw_full[sl].reshape(MT, 128, NCLS).transpose(1, 0, 2)
        )
        in_maps.append(
            {
                "et8": et8,
                "g8": g8,
                "maskI": maskI,
                "posw": posw_c,
                "posb": posb_c,
            }
        )
    return in_maps


def run(embeddings, labels, trace=False, tmpdir=None):
    """Build+run on 8 cores; returns (loss_scalar, BassKernelResults)."""
    from concourse.bass_utils import run_bass_kernel_spmd

    nc = _build_program()
    in_maps = _prep_inputs(embeddings, labels)
    res = run_bass_kernel_spmd(
        nc, in_maps, list(range(NCORES)), trace=trace, tmpdir=tmpdir
    )
    total = 0.0
    for r in res.results:
        ov = r["out_vals"].astype(np.float64)
        total += float((ov[:, :, 0] - np.log(ov[:, :, 1])).sum())
    loss = -total / N * TEMP
    return np.float32(loss), res


def kernel(**inputs) -> np.ndarray:
    loss, _ = run(inputs["embeddings"], inputs["labels"])
    return loss


# revision 9
# speedup vs baseline: 1.0207x; 1.0207x over previous
"""SupCon loss kernel for Trainium2 (8 NeuronCores, SPMD row-sharded), v2.

Math (matches the reference):
  S = (E @ E^T) / T,  T = 0.1
  pos_term_i = mean_{j != i, lab_j = lab_i} S_ij
  lse_i      = logsumexp_{j != i} S_ij
  loss       = -sum_i (pos_term_i - lse_i) / N * T

v2 design (per core c, owning rows c*1024 .. c*1024+1023):
  - E pre-scaled by sqrt(1/T) and quantized to fp8e4 (TRN FP8_EXP4,
    max 240; our values are ~N(0, 3.16), rel loss err ~1e-3 vs 2e-2 gate).
  - Column-rotated E^T per core (own rows first) so the program is pure
    SPMD; stored as 8 segment tiles [128, KC=4, 1024] fp8 + G tile.
  - PE: DoubleRow fp8 matmuls (2 k-subtiles per instruction, 0.5 cyc/row)
    into [128, 2048] psum segs (4 banks, 512-col slices per matmul).
  - Diagonal self-mask via a tiny bf16 matmul (cI)^T @ (-cI) that adds
    -c^2 on the diagonal of the own-block in psum (no DVE pass needed).
  - DVE: one tensor_reduce(max, negate) per seg -> -segmax (exp bias).
  - ACT: one big ACTIVATE(Exp) per seg reading all 4 psum banks, with
    accum_out giving the seg sumexp; per-tile combine rescales segs to
    the row max (online-softmax accounting).
  - pos term: E_local @ (G/2) in fp8 (G = per-class embedding sums) into
    a psum corner before the S loop; host-prepped weights fold in 2/cnt
    and the self-dot correction.
  - Output: per-row [128, MT, 2]: (posacc - posb - rowmax, sumexp).
Host sums the 8 partial outputs -> loss = -total / N * T.
"""

import os
import sys

import numpy as np

for _p in (
    "/root/.axon_site",
    "/root/.axon_site/_ro/trn_rl_repo",
    "/root/.axon_site/_ro/pypackages",
    "/opt/trn_rl_repo",
):
    if os.path.isdir(_p) and _p not in sys.path:
        sys.path.append(_p)

import ml_dtypes

N, D, NCLS, NCORES = 8192, 512, 16, 8
ROWS = N // NCORES        # 1024 rows per core
MT = ROWS // 128          # 8 m-tiles per core
TEMP = 0.1
SCALE = 1.0 / TEMP        # 10.0
KC = D // 128             # 4 k-subtiles
SEG = 2048                # psum segment width (4 banks)
NSEG = N // SEG           # 4
DSEG = 1024               # DMA segment width
NDSEG = N // DSEG         # 8
MASKC = 240.0             # diag mask: -MASKC^2 = -57600 (fp16-safe) on diagonal

USE_DR = True             # DoubleRow fp8 perf mode

_PROG: dict = {}


def _build_program():
    if "nc" in _PROG:
        return _PROG["nc"]

    import concourse.tile as tile
    from concourse import bacc, mybir

    dt = mybir.dt
    Alu = mybir.AluOpType
    Act = mybir.ActivationFunctionType
    f32, bf16, fp8 = dt.float32, dt.bfloat16, dt.float8e4
    f16 = dt.float16
    DR = mybir.MatmulPerfMode.DoubleRow if USE_DR else None

    nc = bacc.Bacc("TRN2", target_bir_lowering=False, debug=False)

    et_d = nc.dram_tensor("et8", [128, KC, N], fp8, kind="ExternalInput").ap()
    g_d = nc.dram_tensor("g8", [128, KC, NCLS], fp8, kind="ExternalInput").ap()
    mask_d = nc.dram_tensor("maskI", [128, 256], bf16, kind="ExternalInput").ap()
    posw_d = nc.dram_tensor("posw", [128, MT, NCLS], f32, kind="ExternalInput").ap()
    posb_d = nc.dram_tensor("posb", [128, MT], f32, kind="ExternalInput").ap()
    out_d = nc.dram_tensor("out_vals", [128, MT, 2], f32, kind="ExternalOutput").ap()

    with tile.TileContext(nc) as tc:
        with (
            tc.tile_pool(name="ets", bufs=1) as ets,
            tc.tile_pool(name="consts", bufs=1) as consts,
            tc.tile_pool(name="small", bufs=2) as small,
            tc.tile_pool(name="cpy", bufs=2) as cpy,
            tc.tile_pool(name="dump", bufs=1) as dump,
            tc.tile_pool(name="acc", bufs=1) as accp,
            tc.tile_pool(name="psum", bufs=2, space="PSUM") as psum,
        ):
            # ---- DMA: G + seg0 first (pos matmuls + first S seg), then rest
            gt = ets.tile([128, KC, NCLS], fp8, name="g8t")
            nc.gpsimd.dma_start(gt[:], g_d[:])
            ets_seg = []
            for s in range(NDSEG):
                st = ets.tile([128, KC, DSEG], fp8, name=f"et_{s}")
                eng = nc.sync if s % 2 == 0 else nc.gpsimd
                eng.dma_start(st[:], et_d[:, :, s * DSEG : (s + 1) * DSEG])
                ets_seg.append(st)

            maskc = consts.tile([128, 256], bf16)
            nc.sync.dma_start(maskc[:], mask_d[:])
            posw = consts.tile([128, MT, NCLS], f32)
            nc.sync.dma_start(posw[:], posw_d[:])
            posb = consts.tile([128, MT], f32)
            nc.sync.dma_start(posb[:], posb_d[:])

            vals = accp.tile([128, MT, 2], f32)
            posc = accp.tile([128, 128], f32)
            zeros = consts.tile([128, SEG], f32)
            nc.vector.memset(zeros[:], 0.0)

            # ---- pos matmuls: C[:, t*16+l] = <e_(t,:), G_l/2>, fp8 1x mode
            ps0 = psum.tile([128, SEG], f32, tag="ps")
            for t in range(MT):
                lt = t * 128
                for k in range(KC):
                    nc.tensor.matmul(
                        ps0[:, t * NCLS : (t + 1) * NCLS],
                        ets_seg[0][:, k, lt : lt + 128],
                        gt[:, k, :],
                        start=(k == 0),
                        stop=(k == KC - 1),
                    )
            nc.vector.tensor_copy(posc[:], ps0[:, 0:128])

            # ---- main loop over m-tiles
            for t in range(MT):
                nm4 = small.tile([128, NSEG], f32, tag="nm")
                nmh = small.tile([128, NSEG, 2], f32, tag="nmh")
                se4 = small.tile([128, NSEG], f32, tag="se")
                lt = t * 128

                for q in range(NSEG):
                    ps = psum.tile([128, SEG], f32, tag="ps")
                    if USE_DR:
                        for a in range(2):
                            lhsT = ets_seg[0][:, 2 * a : 2 * a + 2, lt : lt + 128]
                            for j in range(4):
                                c0 = q * SEG + j * 512
                                s, off = c0 // DSEG, c0 % DSEG
                                rhs = ets_seg[s][:, 2 * a : 2 * a + 2, off : off + 512]
                                nc.tensor.matmul(
                                    ps[:, j * 512 : (j + 1) * 512],
                                    lhsT,
                                    rhs,
                                    start=(a == 0),
                                    stop=(a == 1),
                                    perf_mode=DR,
                                )
                    else:
                        for k in range(KC):
                            lhsT = ets_seg[0][:, k, lt : lt + 128]
                            for j in range(4):
                                c0 = q * SEG + j * 512
                                s, off = c0 // DSEG, c0 % DSEG
                                rhs = ets_seg[s][:, k, off : off + 512]
                                nc.tensor.matmul(
                                    ps[:, j * 512 : (j + 1) * 512],
                                    lhsT,
                                    rhs,
                                    start=(k == 0),
                                    stop=(k == KC - 1),
                                )
                    if q == 0:
                        # add -MASKC^2 on the self-diagonal (cols lt..lt+128)
                        nc.tensor.matmul(
                            ps[:, lt : lt + 128],
                            maskc[:, 0:128],
                            maskc[:, 128:256],
                            start=False,
                            stop=False,
                            skip_group_check=True,
                        )

                    # half-seg maxes overlap the PE fill of the later chunks
                    for h in range(2):
                        nc.vector.tensor_reduce(
                            nmh[:, q, h : h + 1], ps[:, h * 1024 : (h + 1) * 1024],
                            axis=mybir.AxisListType.X, op=Alu.max, negate=True,
                        )
                    nc.vector.tensor_reduce(
                        nm4[:, q : q + 1], nmh[:, q, :],
                        axis=mybir.AxisListType.X, op=Alu.min,
                    )
                    expd = dump.tile([128, SEG], bf16, tag="expd")
                    nc.scalar.activation(
                        expd[:], ps[:], Act.Exp, bias=nm4[:, q : q + 1],
                        scale=1.0, accum_out=se4[:, q : q + 1],
                    )

                # combine: sumexp = sum_q se_q * exp(segmax_q - rowmax)
                negb = small.tile([128, 1], f32, tag="negb")     # -rowmax
                nc.vector.tensor_reduce(
                    negb[:], nm4[:], axis=mybir.AxisListType.X, op=Alu.min
                )
                e4 = small.tile([128, NSEG], f32, tag="e4")
                nc.scalar.activation(
                    e4[:], nm4[:], Act.Exp, bias=negb[:], scale=-1.0
                )
                t4 = small.tile([128, NSEG], f32, tag="t4")
                nc.vector.scalar_tensor_tensor(
                    out=t4[:], in0=se4[:], scalar=1.0, in1=e4[:],
                    op0=Alu.mult, op1=Alu.mult,
                    accum_out=vals[:, t, 1:2],
                )

                pos16 = small.tile([128, NCLS], f32, tag="pos16")
                posacc = small.tile([128, 1], f32, tag="posacc")
                nc.vector.scalar_tensor_tensor(
                    out=pos16[:], in0=posc[:, t * NCLS : (t + 1) * NCLS],
                    scalar=1.0, in1=posw[:, t, :],
                    op0=Alu.mult, op1=Alu.mult,
                    accum_out=posacc[:],
                )
                v1 = small.tile([128, 1], f32, tag="v1")
                nc.vector.tensor_sub(v1[:], posacc[:], posb[:, t : t + 1])
                nc.vector.tensor_add(vals[:, t, 0:1], v1[:], negb[:])

            nc.sync.dma_start(out_d[:], vals[:])

    nc.compile()
    _PROG["nc"] = nc
    return nc


def _prep_inputs(embeddings: np.ndarray, labels: np.ndarray):
    E = np.asarray(embeddings, dtype=np.float32)
    lab = np.asarray(labels).astype(np.int64)
    assert E.shape == (N, D) and lab.shape == (N,)

    # pre-scale by sqrt(1/T) so PSUM dots are already in S-units
    E8 = (E * np.float32(np.sqrt(SCALE))).astype(ml_dtypes.float8_e4m3)
    Ef = E8.astype(np.float64)

    # per-class embedding sums (halved to stay in fp8 range; posw carries 2x)
    G = np.zeros((D, NCLS), np.float64)
    for l in range(NCLS):
        G[:, l] = Ef[lab == l].sum(axis=0)
    G8 = (G / 2.0).astype(ml_dtypes.float8_e4m3)
    g8 = np.ascontiguousarray(G8.reshape(KC, 128, NCLS).transpose(1, 0, 2))

    ET = np.ascontiguousarray(E8.T)               # [D, N] fp8

    cnt = np.bincount(lab, minlength=NCLS).astype(np.float64)
    cnt_i = cnt[lab] - 1.0                        # positives per anchor
    selfdot = (Ef * Ef).sum(axis=1)               # ||e_i||^2 (S-units)
    posb_full = (selfdot / cnt_i).astype(np.float32)
    posw_full = np.zeros((N, NCLS), np.float32)
    posw_full[np.arange(N), lab] = (2.0 / cnt_i).astype(np.float32)

    maskI = np.zeros((128, 256), np.float32)
    maskI[np.arange(128), np.arange(128)] = MASKC
    maskI[np.arange(128), np.arange(128) + 128] = -MASKC
    maskI = maskI.astype(ml_dtypes.bfloat16)

    in_maps = []
    for c in range(NCORES):
        rot = np.roll(ET, -c * ROWS, axis=1)      # own columns first
        et8 = np.ascontiguousarray(
            rot.reshape(KC, 128, N).transpose(1, 0, 2)
        )
        sl = slice(c * ROWS, (c + 1) * ROWS)
        posb_c = np.ascontiguousarray(posb_full[sl].reshape(MT, 128).T)
        posw_c = np.ascontiguousarray(
            posw_full[sl].reshape(MT, 128, NCLS).transpose(1, 0, 2)
        )
        in_maps.append(
            {
                "et8": et8,
                "g8": g8,
                "maskI": maskI,
                "posw": posw_c,
                "posb": posb_c,
            }
        )
    return in_maps


def run(embeddings, labels, trace=False, tmpdir=None):
    """Build+run on 8 cores; returns (loss_scalar, BassKernelResults)."""
    from concourse.bass_utils import run_bass_kernel_spmd

    nc = _build_program()
    in_maps = _prep_inputs(embeddings, labels)
    res = run_bass_kernel_spmd(
        nc, in_maps, list(range(NCORES)), trace=trace, tmpdir=tmpdir
    )
    total = 0.0
    for r in res.results:
        ov = r["out_vals"].astype(np.float64)
        total += float((ov[:, :, 0] - np.log(ov[:, :, 1])).sum())
    loss = -total / N * TEMP
    return np.float32(loss), res


def kernel(**inputs) -> np.ndarray:
    loss, _ = run(inputs["embeddings"], inputs["labels"])
    return loss
